# revision 11
# baseline (speedup 1.0000x reference)
"""Binary KL divergence sum on 8 Trainium2 NeuronCores.

Reference math (per element, summed over all 2**25 elements):
    kl = p*(ln p - ln q) + (1-p)*(ln(1-p) - ln(1-q))

Rewritten with t1 = ln p - ln q, t2 = ln(1-p) - ln(1-q):
    kl = t2 + p*(t1 - t2)
    sum(kl) = sum(t2) + sum(p * (t1 - t2))

Sharding: element axis split evenly across 8 cores. The host packs each
core's share as one "pq" buffer of chunk-interleaved [p-block | q-block]
slabs so each chunk is a single contiguous DMA. Each core computes
per-partition partial sums via PE matmul accumulation; the host sums the
8 * 512 partials.

Per-core pipeline (chunk of [128, 2F], fp32):
  DMA   : pq chunk (p cols :F, q cols F:)             one contiguous DMA
  ACT   : L  = Ln(pq)          -> fp16   (lp | lq   in one instr)
  ACT   : L1 = Ln(1 - pq)      -> fp16   (l1p | l1q in one instr)
  DVE   : p16 = copy(pq[:, :F])                       (fp32 -> fp16, 2x)
  DVE   : t1 = L[:, :F] - L[:, F:]                    (fp16 TT, 2x)
  DVE   : t2 = L1[:, :F] - L1[:, F:]                  (fp16 TT, 2x)
  DVE   : d  = t1 - t2                                (fp16 TT, 2x)
  DVE   : m  = p16 * d                                (fp16 TT, 2x)
  PE    : acc[1,512] += ones^T @ t2 ; ones^T @ m      (PSUM accumulate)

The scalar (ACT) engine is the bottleneck: 4 Ln evaluations per element
pair = 131072 columns = 109.2 us minimum at 1 col/cycle @ 1.2 GHz. The
chunk schedule uses few large chunks to minimize per-ACTIVATE init
overhead (352 cycles each) and semaphore instructions on the ACT queue,
with small chunks at the start (pipeline ramp) and end (short tail).
"""

import numpy as np

import concourse.bass as bass
import concourse.bacc as bacc
import concourse.mybir as mybir
from concourse import bass_utils
from concourse.tile import TileContext

N = 33554432
NCORES = 8
PER = N // NCORES  # 4194304 elements per core per tensor
P = 128

AF = mybir.ActivationFunctionType
OP = mybir.AluOpType
DT = mybir.dt

_NC_CACHE = {}

NRED = 512  # one PSUM bank of fp32: matmul free-dim chunk
# free-dim widths per chunk (per tensor): ramp up, big middle, short tail
CHUNKS = [512, 1024, 2048] + [3584] * 8 + [512]
assert sum(CHUNKS) == PER // P
assert all(w % NRED == 0 for w in CHUNKS)


def _build_nc():
    nc = bacc.Bacc("TRN2", target_bir_lowering=False, debug=False,
                   num_devices=NCORES)
    pq_in = nc.dram_tensor("pq", [2 * PER], DT.float32, kind="ExternalInput")
    out = nc.dram_tensor("partials", [NRED], DT.float32,
                         kind="ExternalOutput")
    out2 = nc.dram_tensor("partials2", [2 * P], DT.float32,
                         kind="ExternalOutput")

    pq_flat = pq_in.ap()
    out_view = out.ap().rearrange("(o n) -> o n", o=1)
    out2_view = out2.ap().rearrange("(p n) -> p n", p=P)

    n_mm = 2 * sum(w // NRED for w in CHUNKS[:-1])

    with TileContext(nc) as tc:
        with (
            tc.tile_pool(name="io32", bufs=4) as io32,
            tc.tile_pool(name="l16", bufs=2) as l16,
            tc.tile_pool(name="tsm", bufs=2) as tsm,
            tc.tile_pool(name="t1d", bufs=1) as t1d,
            tc.tile_pool(name="cst", bufs=1) as cst,
            tc.tile_pool(name="ps", bufs=1, space="PSUM") as psp,
        ):
            ones = cst.tile([P, 1], DT.float16, tag="ones")
            nc.vector.memset(ones[:], 1.0)
            acc = psp.tile([1, NRED], DT.float32, tag="acc")
            osb = cst.tile([1, NRED], DT.float32, tag="osb")

            # Dummy 1-element Ln at t=0 so the ACT table load happens while
            # the first DMA is still in flight. Output goes to osb (live
            # tensor, overwritten later) so DCE keeps it.
            warm = cst.tile([1, 1], DT.float32, tag="warm")
            nc.vector.memset(warm[:], 0.5)
            nc.scalar.activation(osb[0:1, 0:1], warm[:], AF.Ln)

            mm = 0

            def mm_accum(src, w):
                nonlocal mm
                for c in range(w // NRED):
                    nc.tensor.matmul(
                        acc[:, :], ones[:], src[:, c * NRED:(c + 1) * NRED],
                        start=(mm == 0), stop=(mm == n_mm - 1))
                    mm += 1

            accS = cst.tile([P, 2], DT.float32, tag="accS")

            base = 0
            for F in CHUNKS[:-1]:
                pq = io32.tile([P, 2 * F], DT.float32, tag="pq")
                nc.sync.dma_start(
                    pq[:, :],
                    pq_flat[base:base + 2 * P * F].rearrange(
                        "(p f) -> p f", p=P))
                base += 2 * P * F

                L = l16.tile([P, 2 * F], DT.float16, tag="L")
                L1 = l16.tile([P, 2 * F], DT.float16, tag="L1")
                nc.scalar.activation(L[:], pq[:], AF.Ln)
                nc.scalar.activation(L1[:], pq[:], AF.Ln, bias=1.0, scale=-1.0)

                p16 = t1d.tile([P, F], DT.float16, tag="p16")
                nc.vector.tensor_copy(p16[:], pq[:, 0:F])

                t1 = t1d.tile([P, F], DT.float16, tag="t1")
                nc.vector.tensor_tensor(t1[:], L[:, 0:F], L[:, F:2 * F],
                                        OP.subtract)

                t2 = tsm.tile([P, F], DT.float16, tag="t2")
                nc.vector.tensor_tensor(t2[:], L1[:, 0:F], L1[:, F:2 * F],
                                        OP.subtract)

                d = t1d.tile([P, F], DT.float16, tag="d")
                nc.vector.tensor_tensor(d[:], t1[:], t2[:], OP.subtract)

                m = tsm.tile([P, F], DT.float16, tag="m")
                nc.vector.tensor_tensor(m[:], p16[:], d[:], OP.mult)

                mm_accum(t2, F)  # sum(t2)
                mm_accum(m, F)   # sum(p*(t1-t2))

            # Last chunk's input DMA first in Sync program order so it
            # prefetches; then close out the PSUM accumulation while the
            # last chunk is still in flight on ACT/DVE.
            F = CHUNKS[-1]
            pq = io32.tile([P, 2 * F], DT.float32, tag="pq")
            nc.sync.dma_start(
                pq[:, :],
                pq_flat[base:base + 2 * P * F].rearrange("(p f) -> p f", p=P))

            nc.vector.tensor_copy(osb[:], acc[:])
            nc.sync.dma_start(out_view[:], osb[:])

            # Last chunk: sums via DVE accumulators (no PE/PSUM round trip
            # on the critical tail path).
            L = l16.tile([P, 2 * F], DT.float16, tag="L")
            L1 = l16.tile([P, 2 * F], DT.float16, tag="L1")
            nc.scalar.activation(L[:], pq[:], AF.Ln)
            nc.scalar.activation(L1[:], pq[:], AF.Ln, bias=1.0, scale=-1.0)
            p16 = t1d.tile([P, F], DT.float16, tag="p16")
            nc.vector.tensor_copy(p16[:], pq[:, 0:F])
            t1 = t1d.tile([P, F], DT.float16, tag="t1")
            nc.vector.tensor_tensor(t1[:], L[:, 0:F], L[:, F:2 * F],
                                    OP.subtract)
            t2 = tsm.tile([P, F], DT.float16, tag="t2")
            nc.vector.scalar_tensor_tensor(
                t2[:], L1[:, 0:F], 1.0, L1[:, F:2 * F], OP.mult, OP.subtract,
                accum_out=accS[:, 0:1])
            d = t1d.tile([P, F], DT.float16, tag="d")
            nc.vector.tensor_tensor(d[:], t1[:], t2[:], OP.subtract)
            m = tsm.tile([P, F], DT.float16, tag="m")
            nc.vector.scalar_tensor_tensor(
                m[:], p16[:], 1.0, d[:], OP.mult, OP.mult,
                accum_out=accS[:, 1:2])
            nc.sync.dma_start(out2_view[:], accS[:])

    nc.compile()
    return nc


def _get_nc():
    if "nc" not in _NC_CACHE:
        _NC_CACHE["nc"] = _build_nc()
    return _NC_CACHE["nc"]


def _pack_core(p_core, q_core):
    """Interleave per-chunk [p-block | q-block] slabs, each [128, F] flat."""
    buf = np.empty(2 * PER, dtype=np.float32)
    src = 0
    dst = 0
    for F in CHUNKS:
        n = P * F
        blk = buf[dst:dst + 2 * n].reshape(P, 2 * F)
        blk[:, :F] = p_core[src:src + n].reshape(P, F)
        blk[:, F:] = q_core[src:src + n].reshape(P, F)
        src += n
        dst += 2 * n
    return buf


def kernel(input, target, _trace=False):
    input = np.ascontiguousarray(np.asarray(input), dtype=np.float32)
    target = np.ascontiguousarray(np.asarray(target), dtype=np.float32)
    nc = _get_nc()
    in_maps = [
        {"pq": _pack_core(input[c * PER:(c + 1) * PER],
                          target[c * PER:(c + 1) * PER])}
        for c in range(NCORES)
    ]
    res = bass_utils.run_bass_kernel_spmd(
        nc, in_maps, core_ids=list(range(NCORES)), trace=_trace)
    total = np.float64(0.0)
    for c in range(NCORES):
        total += res.results[c]["partials"].astype(np.float64).sum()
        total += res.results[c]["partials2"].astype(np.float64).sum()
    out = np.asarray(total, dtype=np.float32)
    if _trace:
        return out, res
    return out


# revision 12
# speedup vs baseline: 1.1973x; 1.1973x over previous
"""Binary KL divergence sum on 8 Trainium2 NeuronCores.

Reference math (per element, summed over all 2**25 elements):
    kl = p*(ln p - ln q) + (1-p)*(ln(1-p) - ln(1-q))

Rewritten with t1 = ln p - ln q, t2 = ln(1-p) - ln(1-q):
    kl = t2 + p*(t1 - t2)
    sum(kl) = sum(t2) + sum(p * (t1 - t2))

Sharding: element axis split evenly across 8 cores. The host packs each
core's share as one "pq" buffer of chunk-interleaved [p-block | q-block]
slabs so each chunk is a single contiguous DMA. Each core computes
per-partition partial sums via PE matmul accumulation; the host sums the
8 * 512 partials.

Per-core pipeline (chunk of [128, 2F], fp32):
  DMA   : pq chunk (p cols :F, q cols F:)             one contiguous DMA
  ACT   : L  = Ln(pq)          -> fp16   (lp | lq   in one instr)
  ACT   : L1 = Ln(1 - pq)      -> fp16   (l1p | l1q in one instr)
  DVE   : p16 = copy(pq[:, :F])                       (fp32 -> fp16, 2x)
  DVE   : t1 = L[:, :F] - L[:, F:]                    (fp16 TT, 2x)
  DVE   : t2 = L1[:, :F] - L1[:, F:]                  (fp16 TT, 2x)
  DVE   : d  = t1 - t2                                (fp16 TT, 2x)
  DVE   : m  = p16 * d                                (fp16 TT, 2x)
  PE    : acc[1,512] += ones^T @ t2 ; ones^T @ m      (PSUM accumulate)

The scalar (ACT) engine is the bottleneck: 4 Ln evaluations per element
pair = 131072 columns = 109.2 us minimum at 1 col/cycle @ 1.2 GHz
(ACTIVATE rate is dtype-independent; measured cost (N+352)/1.2 ns).
The schedule keeps the ACT queue saturated end to end:
  - ramp-in/ramp-out chunks bound the first-DMA wait and the final DVE
    drain; middle chunks are large to cut per-ACTIVATE init overhead
  - io32 bufs=4 keeps DMA several chunks ahead, riding out the
    DVFS/thermal throttling this part exhibits under sustained load
  - one contiguous DMA per chunk (host-interleaved pq layout) halves
    DMA issues and ACT semaphore waits vs separate p/q transfers
"""

import numpy as np

import concourse.bass as bass
import concourse.bacc as bacc
import concourse.mybir as mybir
from concourse import bass_utils
from concourse.tile import TileContext

N = 33554432
NCORES = 8
PER = N // NCORES  # 4194304 elements per core per tensor
P = 128

AF = mybir.ActivationFunctionType
OP = mybir.AluOpType
DT = mybir.dt

_NC_CACHE = {}

NRED = 512  # one PSUM bank of fp32: matmul free-dim chunk
# free-dim widths per chunk (per tensor): ramp up, big middle, short tail
CHUNKS = [512, 1024, 2048] + [3584] * 8 + [512]
assert sum(CHUNKS) == PER // P
assert all(w % NRED == 0 for w in CHUNKS)


def _build_nc():
    nc = bacc.Bacc("TRN2", target_bir_lowering=False, debug=False,
                   num_devices=NCORES)
    pq_in = nc.dram_tensor("pq", [2 * PER], DT.float32, kind="ExternalInput")
    out = nc.dram_tensor("partials", [NRED], DT.float32,
                         kind="ExternalOutput")

    pq_flat = pq_in.ap()
    out_view = out.ap().rearrange("(o n) -> o n", o=1)

    n_mm = 2 * sum(w // NRED for w in CHUNKS)

    with TileContext(nc) as tc:
        with (
            tc.tile_pool(name="io32", bufs=4) as io32,
            tc.tile_pool(name="l16", bufs=2) as l16,
            tc.tile_pool(name="tsm", bufs=2) as tsm,
            tc.tile_pool(name="t1d", bufs=1) as t1d,
            tc.tile_pool(name="cst", bufs=1) as cst,
            tc.tile_pool(name="ps", bufs=1, space="PSUM") as psp,
        ):
            ones = cst.tile([P, 1], DT.float16, tag="ones")
            nc.vector.memset(ones[:], 1.0)
            acc = psp.tile([1, NRED], DT.float32, tag="acc")
            osb = cst.tile([1, NRED], DT.float32, tag="osb")

            # Dummy 1-element Ln at t=0 so the ACT table load happens while
            # the first DMA is still in flight. Output goes to osb (live
            # tensor, overwritten later) so DCE keeps it.
            warm = cst.tile([1, 1], DT.float32, tag="warm")
            nc.vector.memset(warm[:], 0.5)
            nc.scalar.activation(osb[0:1, 0:1], warm[:], AF.Ln)

            mm = 0

            def mm_accum(src, w):
                nonlocal mm
                for c in range(w // NRED):
                    nc.tensor.matmul(
                        acc[:, :], ones[:], src[:, c * NRED:(c + 1) * NRED],
                        start=(mm == 0), stop=(mm == n_mm - 1))
                    mm += 1

            base = 0
            for F in CHUNKS:
                pq = io32.tile([P, 2 * F], DT.float32, tag="pq")
                nc.sync.dma_start(
                    pq[:, :],
                    pq_flat[base:base + 2 * P * F].rearrange(
                        "(p f) -> p f", p=P))
                base += 2 * P * F

                L = l16.tile([P, 2 * F], DT.float16, tag="L")
                L1 = l16.tile([P, 2 * F], DT.float16, tag="L1")
                nc.scalar.activation(L[:], pq[:], AF.Ln)
                nc.scalar.activation(L1[:], pq[:], AF.Ln, bias=1.0, scale=-1.0)

                p16 = t1d.tile([P, F], DT.float16, tag="p16")
                nc.vector.tensor_copy(p16[:], pq[:, 0:F])

                t1 = t1d.tile([P, F], DT.float16, tag="t1")
                nc.vector.tensor_tensor(t1[:], L[:, 0:F], L[:, F:2 * F],
                                        OP.subtract)

                t2 = tsm.tile([P, F], DT.float16, tag="t2")
                nc.vector.tensor_tensor(t2[:], L1[:, 0:F], L1[:, F:2 * F],
                                        OP.subtract)

                d = t1d.tile([P, F], DT.float16, tag="d")
                nc.vector.tensor_tensor(d[:], t1[:], t2[:], OP.subtract)

                m = tsm.tile([P, F], DT.float16, tag="m")
                nc.vector.tensor_tensor(m[:], p16[:], d[:], OP.mult)

                mm_accum(t2, F)  # sum(t2)
                mm_accum(m, F)   # sum(p*(t1-t2))

            nc.vector.tensor_copy(osb[:], acc[:])
            nc.sync.dma_start(out_view[:], osb[:])

    nc.compile()
    return nc


def _get_nc():
    if "nc" not in _NC_CACHE:
        _NC_CACHE["nc"] = _build_nc()
    return _NC_CACHE["nc"]


def _pack_core(p_core, q_core):
    """Interleave per-chunk [p-block | q-block] slabs, each [128, F] flat."""
    buf = np.empty(2 * PER, dtype=np.float32)
    src = 0
    dst = 0
    for F in CHUNKS:
        n = P * F
        blk = buf[dst:dst + 2 * n].reshape(P, 2 * F)
        blk[:, :F] = p_core[src:src + n].reshape(P, F)
        blk[:, F:] = q_core[src:src + n].reshape(P, F)
        src += n
        dst += 2 * n
    return buf


def kernel(input, target, _trace=False):
    input = np.ascontiguousarray(np.asarray(input), dtype=np.float32)
    target = np.ascontiguousarray(np.asarray(target), dtype=np.float32)
    nc = _get_nc()
    in_maps = [
        {"pq": _pack_core(input[c * PER:(c + 1) * PER],
                          target[c * PER:(c + 1) * PER])}
        for c in range(NCORES)
    ]
    res = bass_utils.run_bass_kernel_spmd(
        nc, in_maps, core_ids=list(range(NCORES)), trace=_trace)
    total = np.float64(0.0)
    for c in range(NCORES):
        total += res.results[c]["partials"].astype(np.float64).sum()
    out = np.asarray(total, dtype=np.float32)
    if _trace:
        return out, res
    return out
